# revision 4
# baseline (speedup 1.0000x reference)
"""Trainium2 Bass kernel for MultiHeadLatentAttention (B=4, S=8192, E=2048,
H=16, latent=head_dim=128), SPMD over 8 NeuronCores.

Math (reference):
    q = rope(X_q @ Wq + bq); k = rope(X_k @ Wk + bk); v = X_v @ Wv + bv
    reshape folds seq into heads: q[b,h,s',d] = q_lat[b, 16*s'+h, d], S'=512
    attn per (b,h): softmax(q k^T / sqrt(128)) @ v   -> out @ Wo + bo

Key transforms done on the host (exact, no approximation):
  * rope here is position-independent (freqs have a singleton seq axis), so
    rope(x) == x @ R for a fixed 128x128 2-diagonal matrix R. We fold R (and
    the 1/sqrt(128) score scale) into Wq / Wk.
  * mask is all ones -> no-op.
  * Sharding: 2 heads per core x all 4 batches: each core projects exactly the
    input rows its heads need (zero redundant FLOPs), runs 8 [512x512]
    attentions, and computes a partial out-projection over its 256 latent
    channels.  Host sums the 8 partials (+ bo).

Device layout per core:
  qT/kT/vT:  [128 latent, 4096 rows]  (rows grouped by (b, head))
  scores^T:  [128 kpos, 512 qpos] tiles -> exp on ACT -> PV and column sums
  via PE matmuls (ones-vector trick for the softmax denominator).
"""

import os

import numpy as np

import concourse.bass as bass
import concourse.mybir as mybir
import concourse.tile as tile
from concourse import bacc
from concourse.bass_utils import run_bass_kernel_spmd
from concourse.masks import make_identity

B, S, E, H, HD = 4, 8192, 2048, 16, 128
SP = S // H            # 512 folded sequence length
NCORES = 8
HPC = H // NCORES      # heads per core = 2
NG = B * HPC           # attention groups per core = 8
ROWS = NG * SP         # projection rows per core = 4096
OROWS = B * SP         # output rows = 2048
KC = E // 128          # contraction chunks = 16
JT = SP // 128         # 128-blocks per group = 4
F32 = mybir.dt.float32

# matmul mode: 'f32' (full precision, 4 cyc/row), 'f32r' (1 cyc/row, relaxed
# internal rounding), 'bf16' (casts inputs, halves DMA bytes).
MM_MODE = os.environ.get("MLA_MM_MODE", "f32r")

_CACHE = {}
LAST_RESULTS = None  # BassKernelResults of the most recent run (for profiling)


def _build(mm_mode, with_bias):
    io_dt = mybir.dt.bfloat16 if mm_mode == "bf16" else F32
    sb_dt = io_dt

    def mm(ap):
        return ap.bitcast(mybir.dt.float32r) if mm_mode == "f32r" else ap

    nc = bacc.Bacc("TRN2", target_bir_lowering=False, debug=False,
                   num_devices=NCORES)
    xq = nc.dram_tensor("xq", [E, ROWS], io_dt, kind="ExternalInput")
    xk = nc.dram_tensor("xk", [E, ROWS], io_dt, kind="ExternalInput")
    xv = nc.dram_tensor("xv", [E, ROWS], io_dt, kind="ExternalInput")
    wq = nc.dram_tensor("wq", [E, HD], io_dt, kind="ExternalInput")
    wk = nc.dram_tensor("wk", [E, HD], io_dt, kind="ExternalInput")
    wv = nc.dram_tensor("wv", [E, HD], io_dt, kind="ExternalInput")
    wo = nc.dram_tensor("wo", [HPC * HD, E], io_dt, kind="ExternalInput")
    if with_bias:
        bqkv = nc.dram_tensor("bqkv", [3, HD], F32, kind="ExternalInput")
    out = nc.dram_tensor("out", [OROWS, E], F32, kind="ExternalOutput")

    with tile.TileContext(nc) as tc:
        with tc.tile_pool(name="persist", bufs=1) as persist:
            qT = persist.tile([128, ROWS], sb_dt)
            kT = persist.tile([128, ROWS], sb_dt)
            # vN[kpos%128, g, j, d] = v value at k position g*512+j*128+kpos
            vN = persist.tile([128, NG, JT, HD], sb_dt)
            AT = persist.tile([128, HPC, OROWS], sb_dt)
            # all-ones stationary: the denominator matmul ones^T @ E yields the
            # column sums replicated across all 128 output partitions, which
            # sidesteps any cross-partition broadcast for the normalization.
            ones_t = persist.tile([128, 128], sb_dt)
            nc.gpsimd.memset(ones_t[:], 1.0)
            ident = persist.tile([128, 128], sb_dt)
            make_identity(nc, ident[:])
            if with_bias:
                bias_sb = persist.tile([128, 3], F32)
                nc.sync.dma_start(bias_sb[:], bqkv[:].rearrange("t l -> l t"))

            # ---- Phase A: latent projections (out in [latent, rows] layout),
            # ---- Phase B: transpose v into [kpos, d] layout.
            with tc.tile_pool(name="pa", bufs=2) as pa, \
                 tc.tile_pool(name="paw", bufs=1) as paw, \
                 tc.tile_pool(name="pav", bufs=1) as pav, \
                 tc.tile_pool(name="psa", bufs=4, space="PSUM") as psa, \
                 tc.tile_pool(name="pstr", bufs=4, space="PSUM") as pstr:
                w_sb = {}
                for name, w in (("q", wq), ("k", wk), ("v", wv)):
                    t = paw.tile([128, KC, HD], io_dt, tag=f"w_{name}")
                    nc.sync.dma_start(t[:], w[:].rearrange("(c p) l -> p c l", p=128))
                    w_sb[name] = t
                vT = pav.tile([128, ROWS], sb_dt)
                dsts = {"q": qT, "k": kT, "v": vT}
                srcs = {"q": xq, "k": xk, "v": xv}
                for ti, name in enumerate(("q", "k", "v")):
                    xr = srcs[name][:].rearrange("(c p) r -> p c r", p=128)
                    for m in range(ROWS // 512):
                        xm = pa.tile([128, KC, 512], io_dt, tag="xm")
                        nc.sync.dma_start(xm[:], xr[:, :, m * 512:(m + 1) * 512])
                        ps = psa.tile([128, 512], F32, tag="proj")
                        for c in range(KC):
                            nc.tensor.matmul(ps[:], mm(w_sb[name][:, c]),
                                             mm(xm[:, c]),
                                             start=(c == 0), stop=(c == KC - 1))
                        dst = dsts[name][:, m * 512:(m + 1) * 512]
                        if with_bias:
                            nc.vector.tensor_scalar_add(dst, ps[:],
                                                        bias_sb[:, ti:ti + 1])
                        else:
                            nc.vector.tensor_copy(dst, ps[:])
                for g in range(NG):
                    for j in range(JT):
                        pt = pstr.tile([128, 128], sb_dt, tag="tr")
                        base = g * SP + j * 128
                        nc.tensor.transpose(pt[:], vT[:, base:base + 128], ident[:])
                        nc.vector.tensor_copy(vN[:, g, j], pt[:])

            # ---- Phase C: attention per (batch, head) group.
            with tc.tile_pool(name="pc", bufs=2) as pc, \
                 tc.tile_pool(name="pss", bufs=4, space="PSUM") as pss, \
                 tc.tile_pool(name="pssum", bufs=2, space="PSUM") as pssum, \
                 tc.tile_pool(name="pso", bufs=2, space="PSUM") as pso:
                for g in range(NG):
                    b, hl = divmod(g, HPC)
                    Esb = pc.tile([128, JT, SP], sb_dt, tag="E")
                    for j in range(JT):
                        sp = pss.tile([128, SP], F32, tag="S")
                        base = g * SP + j * 128
                        nc.tensor.matmul(sp[:], mm(kT[:, base:base + 128]),
                                         mm(qT[:, g * SP:(g + 1) * SP]),
                                         start=True, stop=True)
                        nc.scalar.activation(Esb[:, j], sp[:],
                                             mybir.ActivationFunctionType.Exp)
                    sum_ps = pssum.tile([128, SP], F32, tag="sum")
                    for j in range(JT):
                        nc.tensor.matmul(sum_ps[:], mm(ones_t[:]), mm(Esb[:, j]),
                                         start=(j == 0), stop=(j == JT - 1))
                    o_ps = pso.tile([128, SP], F32, tag="O")
                    for j in range(JT):
                        nc.tensor.matmul(o_ps[:], mm(vN[:, g, j]), mm(Esb[:, j]),
                                         start=(j == 0), stop=(j == JT - 1))
                    # softmax denominator: reciprocal, then scale PV while
                    # draining its PSUM bank.
                    rec_b = pc.tile([128, SP], F32, tag="recb")
                    nc.vector.reciprocal(rec_b[:], sum_ps[:])
                    nc.vector.tensor_tensor(AT[:, hl, b * SP:(b + 1) * SP],
                                            o_ps[:], rec_b[:],
                                            op=mybir.AluOpType.mult)

            # ---- Phase D: partial out-projection (256 latent channels).
            with tc.tile_pool(name="pd", bufs=3) as pd, \
                 tc.tile_pool(name="pdw", bufs=1) as pdw, \
                 tc.tile_pool(name="psd", bufs=4, space="PSUM") as psd:
                wo_sb = pdw.tile([128, HPC, E], io_dt)
                nc.sync.dma_start(wo_sb[:],
                                  wo[:].rearrange("(h p) e -> p h e", p=128))
                for rt in range(OROWS // 128):
                    for n in range(E // 512):
                        ps = psd.tile([128, 512], F32, tag="od")
                        for hl in range(HPC):
                            nc.tensor.matmul(ps[:],
                                             mm(AT[:, hl, rt * 128:(rt + 1) * 128]),
                                             mm(wo_sb[:, hl, n * 512:(n + 1) * 512]),
                                             start=(hl == 0), stop=(hl == HPC - 1))
                        ot = pd.tile([128, 512], F32, tag="ot")
                        nc.vector.tensor_copy(ot[:], ps[:])
                        nc.sync.dma_start(
                            out[rt * 128:(rt + 1) * 128, n * 512:(n + 1) * 512],
                            ot[:])

    nc.compile()
    return nc


def _rope_matrix():
    h2 = HD // 2
    freqs = 1.0 / (10000.0 ** (np.arange(0, HD, 2, dtype=np.float64) / HD))
    sin, cos = np.sin(freqs), np.cos(freqs)
    R = np.zeros((HD, HD), np.float64)
    i = np.arange(h2)
    R[i, i] = cos
    R[i + h2, i] = -sin
    R[i + h2, i + h2] = cos
    R[i, i + h2] = sin
    return R


def kernel(query, key, value, attn_mask, Wq, bq, Wk, bk, Wv, bv, Wo, bo,
           _trace=False):
    global LAST_RESULTS
    mm_mode = MM_MODE
    io_np = np.dtype("float32")
    if mm_mode == "bf16":
        import ml_dtypes
        io_np = np.dtype(ml_dtypes.bfloat16)

    R = _rope_matrix()
    scale = 1.0 / np.sqrt(np.float64(HD))
    wq_eff = (Wq.astype(np.float64) @ R * scale).astype(io_np)
    wk_eff = (Wk.astype(np.float64) @ R).astype(io_np)
    wv_eff = Wv.astype(io_np)
    bq_eff = (bq.astype(np.float64) @ R * scale).astype(np.float32)
    bk_eff = (bk.astype(np.float64) @ R).astype(np.float32)
    bv_eff = bv.astype(np.float32)
    with_bias = bool(np.any(bq_eff) or np.any(bk_eff) or np.any(bv_eff))

    key_ = (mm_mode, with_bias)
    if key_ not in _CACHE:
        _CACHE[key_] = _build(mm_mode, with_bias)
    nc = _CACHE[key_]

    # [B,S,E] -> [E, B, H, SP]; s = s'*H + h so reshape(B, SP, H, E) puts the
    # folded position s' on axis 1 and the head on axis 2.
    def fold(x):
        return np.ascontiguousarray(
            x.reshape(B, SP, H, E).transpose(3, 0, 2, 1).astype(io_np))

    fq, fk, fv = fold(query), fold(key), fold(value)
    wo_r = Wo.reshape(H, HD, E)

    in_maps = []
    for c in range(NCORES):
        h0 = HPC * c
        m = {
            "xq": fq[:, :, h0:h0 + HPC, :].reshape(E, ROWS),
            "xk": fk[:, :, h0:h0 + HPC, :].reshape(E, ROWS),
            "xv": fv[:, :, h0:h0 + HPC, :].reshape(E, ROWS),
            "wq": wq_eff, "wk": wk_eff, "wv": wv_eff,
            "wo": np.ascontiguousarray(
                wo_r[h0:h0 + HPC].reshape(HPC * HD, E)).astype(io_np),
        }
        if with_bias:
            m["bqkv"] = np.stack([bq_eff, bk_eff, bv_eff])
        in_maps.append(m)

    kwargs = {}
    if _trace:
        kwargs = dict(trace=True, trace_cores=list(range(NCORES)))
    res = run_bass_kernel_spmd(nc, in_maps, core_ids=list(range(NCORES)),
                               **kwargs)
    LAST_RESULTS = res

    total = res.results[0]["out"].astype(np.float64)
    for c in range(1, NCORES):
        total += res.results[c]["out"]
    total += bo.astype(np.float64)
    return total.reshape(B, SP, E).astype(np.float32)


# revision 6
# speedup vs baseline: 1.3149x; 1.3149x over previous
"""Trainium2 Bass kernel for MultiHeadLatentAttention (B=4, S=8192, E=2048,
H=16, latent=head_dim=128), SPMD over 8 NeuronCores.

Math (reference):
    q = rope(X_q @ Wq + bq); k = rope(X_k @ Wk + bk); v = X_v @ Wv + bv
    reshape folds seq into heads: q[b,h,s',d] = q_lat[b, 16*s'+h, d], S'=512
    attn per (b,h): softmax(q k^T / sqrt(128)) @ v   -> out @ Wo + bo

Key transforms done on the host (exact, no approximation):
  * rope here is position-independent (freqs have a singleton seq axis), so
    rope(x) == x @ R for a fixed 128x128 2-diagonal matrix R. We fold R (and
    the 1/sqrt(128) score scale) into Wq / Wk.
  * mask is all ones -> no-op.
  * Sharding: 2 heads per core x all 4 batches: each core projects exactly the
    input rows its heads need (zero redundant FLOPs), runs 8 [512x512]
    attentions, and computes a partial out-projection over its 256 latent
    channels.  Host sums the 8 partials (+ bo).

Device layout per core:
  qT/kT/vT:  [128 latent, 4096 rows]  (rows grouped by (b, head))
  scores^T:  [128 kpos, 512 qpos] tiles -> exp on ACT -> PV and column sums
  via PE matmuls (ones-vector trick for the softmax denominator).
"""

import os

import numpy as np

import concourse.bass as bass
import concourse.mybir as mybir
import concourse.tile as tile
from concourse import bacc
from concourse.bass_utils import run_bass_kernel_spmd
from concourse.masks import make_identity

B, S, E, H, HD = 4, 8192, 2048, 16, 128
SP = S // H            # 512 folded sequence length
NCORES = 8
HPC = H // NCORES      # heads per core = 2
NG = B * HPC           # attention groups per core = 8
ROWS = NG * SP         # projection rows per core = 4096
OROWS = B * SP         # output rows = 2048
KC = E // 128          # contraction chunks = 16
JT = SP // 128         # 128-blocks per group = 4
F32 = mybir.dt.float32

# matmul mode: 'f32' (full precision, 4 cyc/row), 'f32r' (1 cyc/row, relaxed
# internal rounding), 'bf16' (casts inputs, halves DMA bytes).
MM_MODE = os.environ.get("MLA_MM_MODE", "f32r")

_CACHE = {}
LAST_RESULTS = None  # BassKernelResults of the most recent run (for profiling)


def _build(mm_mode, with_bias):
    # dtype of matmul operands: the BIR verifier requires every producer of an
    # fp32r matmul input to declare float32r output (rounding on write), so in
    # f32r mode the DRAM inputs and SBUF compute tiles are float32r end-to-end.
    io_dt = {"bf16": mybir.dt.bfloat16, "f32r": mybir.dt.float32r,
             "f32": F32}[mm_mode]
    sb_dt = io_dt

    def mm(ap):
        return ap

    nc = bacc.Bacc("TRN2", target_bir_lowering=False, debug=False,
                   num_devices=NCORES)
    xq = nc.dram_tensor("xq", [E, ROWS], io_dt, kind="ExternalInput")
    xk = nc.dram_tensor("xk", [E, ROWS], io_dt, kind="ExternalInput")
    xv = nc.dram_tensor("xv", [E, ROWS], io_dt, kind="ExternalInput")
    wq = nc.dram_tensor("wq", [E, HD], io_dt, kind="ExternalInput")
    wk = nc.dram_tensor("wk", [E, HD], io_dt, kind="ExternalInput")
    wv = nc.dram_tensor("wv", [E, HD], io_dt, kind="ExternalInput")
    wo = nc.dram_tensor("wo", [HPC * HD, E], io_dt, kind="ExternalInput")
    if with_bias:
        bqkv = nc.dram_tensor("bqkv", [3, HD], F32, kind="ExternalInput")
    out = nc.dram_tensor("out", [OROWS, E], F32, kind="ExternalOutput")

    with tile.TileContext(nc) as tc:
        with tc.tile_pool(name="persist", bufs=1) as persist:
            qT = persist.tile([128, ROWS], sb_dt)
            kT = persist.tile([128, ROWS], sb_dt)
            # vN[kpos%128, g, j, d] = v value at k position g*512+j*128+kpos
            vN = persist.tile([128, NG, JT, HD], sb_dt)
            AT = persist.tile([128, HPC, OROWS], sb_dt)
            # all-ones stationary: the denominator matmul ones^T @ E yields the
            # column sums replicated across all 128 output partitions, which
            # sidesteps any cross-partition broadcast for the normalization.
            # memset/affine_select only handle plain dtypes, so build in f32
            # and convert (float32r conversion = rounding on DVE write).
            ones_t = persist.tile([128, 128], sb_dt)
            ident = persist.tile([128, 128], sb_dt)
            if sb_dt == F32:
                nc.gpsimd.memset(ones_t[:], 1.0)
                make_identity(nc, ident[:])
            else:
                scratch = persist.tile([128, 128], F32)
                nc.gpsimd.memset(scratch[:], 1.0)
                nc.vector.tensor_copy(ones_t[:], scratch[:])
                make_identity(nc, scratch[:])
                nc.vector.tensor_copy(ident[:], scratch[:])
            if with_bias:
                bias_sb = persist.tile([128, 3], F32)
                nc.sync.dma_start(bias_sb[:], bqkv[:].rearrange("t l -> l t"))

            # ---- Phase A: latent projections (out in [latent, rows] layout),
            # ---- Phase B: transpose v into [kpos, d] layout.
            with tc.tile_pool(name="pa", bufs=2) as pa, \
                 tc.tile_pool(name="paw", bufs=1) as paw, \
                 tc.tile_pool(name="pav", bufs=1) as pav, \
                 tc.tile_pool(name="psa", bufs=4, space="PSUM") as psa, \
                 tc.tile_pool(name="pstr", bufs=4, space="PSUM") as pstr:
                w_sb = {}
                for name, w in (("q", wq), ("k", wk), ("v", wv)):
                    t = paw.tile([128, KC, HD], io_dt, tag=f"w_{name}")
                    nc.sync.dma_start(t[:], w[:].rearrange("(c p) l -> p c l", p=128))
                    w_sb[name] = t
                vT = pav.tile([128, ROWS], sb_dt)
                dsts = {"q": qT, "k": kT, "v": vT}
                srcs = {"q": xq, "k": xk, "v": xv}
                for ti, name in enumerate(("q", "k", "v")):
                    xr = srcs[name][:].rearrange("(c p) r -> p c r", p=128)
                    for m in range(ROWS // 512):
                        xm = pa.tile([128, KC, 512], io_dt, tag="xm")
                        nc.sync.dma_start(xm[:], xr[:, :, m * 512:(m + 1) * 512])
                        ps = psa.tile([128, 512], F32, tag="proj")
                        for c in range(KC):
                            nc.tensor.matmul(ps[:], mm(w_sb[name][:, c]),
                                             mm(xm[:, c]),
                                             start=(c == 0), stop=(c == KC - 1))
                        dst = dsts[name][:, m * 512:(m + 1) * 512]
                        if with_bias:
                            nc.vector.tensor_scalar_add(dst, ps[:],
                                                        bias_sb[:, ti:ti + 1])
                        else:
                            nc.vector.tensor_copy(dst, ps[:])
                for g in range(NG):
                    for j in range(JT):
                        pt = pstr.tile([128, 128], sb_dt, tag="tr")
                        base = g * SP + j * 128
                        nc.tensor.transpose(pt[:], vT[:, base:base + 128], ident[:])
                        nc.vector.tensor_copy(vN[:, g, j], pt[:])

            # ---- Phase C: attention per (batch, head) group.
            with tc.tile_pool(name="pc", bufs=2) as pc, \
                 tc.tile_pool(name="pss", bufs=4, space="PSUM") as pss, \
                 tc.tile_pool(name="pssum", bufs=2, space="PSUM") as pssum, \
                 tc.tile_pool(name="pso", bufs=2, space="PSUM") as pso:
                for g in range(NG):
                    b, hl = divmod(g, HPC)
                    Esb = pc.tile([128, JT, SP], sb_dt, tag="E")
                    for j in range(JT):
                        sp = pss.tile([128, SP], F32, tag="S")
                        base = g * SP + j * 128
                        nc.tensor.matmul(sp[:], mm(kT[:, base:base + 128]),
                                         mm(qT[:, g * SP:(g + 1) * SP]),
                                         start=True, stop=True)
                        nc.scalar.activation(Esb[:, j], sp[:],
                                             mybir.ActivationFunctionType.Exp)
                    sum_ps = pssum.tile([128, SP], F32, tag="sum")
                    for j in range(JT):
                        nc.tensor.matmul(sum_ps[:], mm(ones_t[:]), mm(Esb[:, j]),
                                         start=(j == 0), stop=(j == JT - 1))
                    o_ps = pso.tile([128, SP], F32, tag="O")
                    for j in range(JT):
                        nc.tensor.matmul(o_ps[:], mm(vN[:, g, j]), mm(Esb[:, j]),
                                         start=(j == 0), stop=(j == JT - 1))
                    # softmax denominator: reciprocal, then scale PV while
                    # draining its PSUM bank.
                    rec_b = pc.tile([128, SP], F32, tag="recb")
                    nc.vector.reciprocal(rec_b[:], sum_ps[:])
                    nc.vector.tensor_tensor(AT[:, hl, b * SP:(b + 1) * SP],
                                            o_ps[:], rec_b[:],
                                            op=mybir.AluOpType.mult)

            # ---- Phase D: partial out-projection (256 latent channels).
            with tc.tile_pool(name="pd", bufs=3) as pd, \
                 tc.tile_pool(name="pdw", bufs=1) as pdw, \
                 tc.tile_pool(name="psd", bufs=4, space="PSUM") as psd:
                wo_sb = pdw.tile([128, HPC, E], io_dt)
                nc.sync.dma_start(wo_sb[:],
                                  wo[:].rearrange("(h p) e -> p h e", p=128))
                for rt in range(OROWS // 128):
                    for n in range(E // 512):
                        ps = psd.tile([128, 512], F32, tag="od")
                        for hl in range(HPC):
                            nc.tensor.matmul(ps[:],
                                             mm(AT[:, hl, rt * 128:(rt + 1) * 128]),
                                             mm(wo_sb[:, hl, n * 512:(n + 1) * 512]),
                                             start=(hl == 0), stop=(hl == HPC - 1))
                        ot = pd.tile([128, 512], F32, tag="ot")
                        nc.vector.tensor_copy(ot[:], ps[:])
                        nc.sync.dma_start(
                            out[rt * 128:(rt + 1) * 128, n * 512:(n + 1) * 512],
                            ot[:])

    nc.compile()
    return nc


def _rope_matrix():
    h2 = HD // 2
    freqs = 1.0 / (10000.0 ** (np.arange(0, HD, 2, dtype=np.float64) / HD))
    sin, cos = np.sin(freqs), np.cos(freqs)
    R = np.zeros((HD, HD), np.float64)
    i = np.arange(h2)
    R[i, i] = cos
    R[i + h2, i] = -sin
    R[i + h2, i + h2] = cos
    R[i, i + h2] = sin
    return R


def kernel(query, key, value, attn_mask, Wq, bq, Wk, bk, Wv, bv, Wo, bo,
           _trace=False):
    global LAST_RESULTS
    mm_mode = MM_MODE
    io_np = np.dtype("float32")
    if mm_mode == "bf16":
        import ml_dtypes
        io_np = np.dtype(ml_dtypes.bfloat16)

    R = _rope_matrix()
    scale = 1.0 / np.sqrt(np.float64(HD))
    wq_eff = (Wq.astype(np.float64) @ R * scale).astype(io_np)
    wk_eff = (Wk.astype(np.float64) @ R).astype(io_np)
    wv_eff = Wv.astype(io_np)
    bq_eff = (bq.astype(np.float64) @ R * scale).astype(np.float32)
    bk_eff = (bk.astype(np.float64) @ R).astype(np.float32)
    bv_eff = bv.astype(np.float32)
    with_bias = bool(np.any(bq_eff) or np.any(bk_eff) or np.any(bv_eff))

    key_ = (mm_mode, with_bias)
    if key_ not in _CACHE:
        _CACHE[key_] = _build(mm_mode, with_bias)
    nc = _CACHE[key_]

    # [B,S,E] -> [E, B, H, SP]; s = s'*H + h so reshape(B, SP, H, E) puts the
    # folded position s' on axis 1 and the head on axis 2.
    def fold(x):
        return np.ascontiguousarray(
            x.reshape(B, SP, H, E).transpose(3, 0, 2, 1).astype(io_np))

    fq, fk, fv = fold(query), fold(key), fold(value)
    wo_r = Wo.reshape(H, HD, E)

    in_maps = []
    for c in range(NCORES):
        h0 = HPC * c
        m = {
            "xq": fq[:, :, h0:h0 + HPC, :].reshape(E, ROWS),
            "xk": fk[:, :, h0:h0 + HPC, :].reshape(E, ROWS),
            "xv": fv[:, :, h0:h0 + HPC, :].reshape(E, ROWS),
            "wq": wq_eff, "wk": wk_eff, "wv": wv_eff,
            "wo": np.ascontiguousarray(
                wo_r[h0:h0 + HPC].reshape(HPC * HD, E)).astype(io_np),
        }
        if with_bias:
            m["bqkv"] = np.stack([bq_eff, bk_eff, bv_eff])
        in_maps.append(m)

    kwargs = {}
    if _trace:
        kwargs = dict(trace=True, trace_cores=list(range(NCORES)))
    res = run_bass_kernel_spmd(nc, in_maps, core_ids=list(range(NCORES)),
                               **kwargs)
    LAST_RESULTS = res

    total = res.results[0]["out"].astype(np.float64)
    for c in range(1, NCORES):
        total += res.results[c]["out"]
    total += bo.astype(np.float64)
    return total.reshape(B, SP, E).astype(np.float32)


# revision 14
# speedup vs baseline: 2.0851x; 1.5858x over previous
"""Trainium2 Bass kernel for MultiHeadLatentAttention (B=4, S=8192, E=2048,
H=16, latent=head_dim=128), SPMD over 8 NeuronCores.

Math (reference):
    q = rope(X_q @ Wq + bq); k = rope(X_k @ Wk + bk); v = X_v @ Wv + bv
    reshape folds seq into heads: q[b,h,s',d] = q_lat[b, 16*s'+h, d], S'=512
    attn per (b,h): softmax(q k^T / sqrt(128)) @ v   -> out @ Wo + bo

Key transforms done on the host (exact, no approximation):
  * rope here is position-independent (freqs have a singleton seq axis), so
    rope(x) == x @ R for a fixed 128x128 2-diagonal matrix R. We fold R (and
    the 1/sqrt(128) score scale) into Wq / Wk.
  * mask is all ones -> no-op.
  * Sharding: 2 heads per core x all 4 batches: each core projects exactly the
    input rows its heads need (zero redundant FLOPs), runs 8 [512x512]
    attentions, and computes a partial out-projection over its 256 latent
    channels.  Host sums the 8 partials (+ bo).

Device layout per core:
  qT/kT/vT:  [128 latent, 4096 rows]  (rows grouped by (b, head))
  scores^T:  [128 kpos, 512 qpos] tiles -> exp on ACT -> PV and column sums
  via PE matmuls (ones-vector trick for the softmax denominator).
"""

import os

import numpy as np

import concourse.bass as bass
import concourse.mybir as mybir
import concourse.tile as tile
from concourse import bacc
from concourse.bass_utils import run_bass_kernel_spmd
from concourse.masks import make_identity

B, S, E, H, HD = 4, 8192, 2048, 16, 128
SP = S // H            # 512 folded sequence length
NCORES = 8
HPC = H // NCORES      # heads per core = 2
NG = B * HPC           # attention groups per core = 8
ROWS = NG * SP         # projection rows per core = 4096
OROWS = B * SP         # output rows = 2048
KC = E // 128          # contraction chunks = 16
JT = SP // 128         # 128-blocks per group = 4
F32 = mybir.dt.float32

# matmul mode: 'f32' (full precision, 4 cyc/row), 'f32r' (1 cyc/row, relaxed
# internal rounding), 'bf16' (casts inputs, halves DMA bytes).
MM_MODE = os.environ.get("MLA_MM_MODE", "f32r")

_CACHE = {}
LAST_RESULTS = None  # BassKernelResults of the most recent run (for profiling)


def _build(mm_mode, with_bias):
    # dtype of matmul operands: the BIR verifier requires every producer of an
    # fp32r matmul input to declare float32r output (rounding on write), so in
    # f32r mode the DRAM inputs and SBUF compute tiles are float32r end-to-end.
    io_dt = {"bf16": mybir.dt.bfloat16, "f32r": mybir.dt.float32r,
             "f32": F32}[mm_mode]
    sb_dt = io_dt

    def mm(ap):
        return ap

    nc = bacc.Bacc("TRN2", target_bir_lowering=False, debug=False,
                   num_devices=NCORES)
    xq = nc.dram_tensor("xq", [E, ROWS], io_dt, kind="ExternalInput")
    xk = nc.dram_tensor("xk", [E, ROWS], io_dt, kind="ExternalInput")
    xv = nc.dram_tensor("xv", [E, ROWS], io_dt, kind="ExternalInput")
    # weights arrive pre-laid-out for SBUF ([128 partitions first]) so each
    # load is one DMA with 8KB-contiguous partition lines.
    wq = nc.dram_tensor("wq", [128, KC * HD], io_dt, kind="ExternalInput")
    wk = nc.dram_tensor("wk", [128, KC * HD], io_dt, kind="ExternalInput")
    wv = nc.dram_tensor("wv", [128, KC * HD], io_dt, kind="ExternalInput")
    wo = nc.dram_tensor("wo", [128, HPC * E], io_dt, kind="ExternalInput")
    if with_bias:
        bqkv = nc.dram_tensor("bqkv", [3, HD], F32, kind="ExternalInput")
    out = nc.dram_tensor("out", [OROWS, E], F32, kind="ExternalOutput")

    with tile.TileContext(nc) as tc:
        with tc.tile_pool(name="persist", bufs=1) as persist:
            qT = persist.tile([128, ROWS], sb_dt)
            kT = persist.tile([128, ROWS], sb_dt)
            # vN[kpos%128, g, j, d] = v value at k position g*512+j*128+kpos
            vN = persist.tile([128, NG, JT, HD], sb_dt)
            AT = persist.tile([128, HPC, OROWS], sb_dt)
            # all-ones stationary: the denominator matmul ones^T @ E yields the
            # column sums replicated across all 128 output partitions, which
            # sidesteps any cross-partition broadcast for the normalization.
            # memset/affine_select only handle plain dtypes, so build in f32
            # and convert (float32r conversion = rounding on DVE write).
            ones_t = persist.tile([128, 128], sb_dt)
            ident = persist.tile([128, 128], sb_dt)
            if sb_dt == F32:
                nc.gpsimd.memset(ones_t[:], 1.0)
                make_identity(nc, ident[:])
            else:
                scratch = persist.tile([128, 128], F32)
                nc.gpsimd.memset(scratch[:], 1.0)
                nc.vector.tensor_copy(ones_t[:], scratch[:])
                make_identity(nc, scratch[:])
                nc.vector.tensor_copy(ident[:], scratch[:])
            if with_bias:
                bias_sb = persist.tile([128, 3], F32)
                nc.sync.dma_start(bias_sb[:], bqkv[:].rearrange("t l -> l t"))

            # ---- Phase A: latent projections (out in [latent, rows] layout).
            # E-chunk-outer loop: each x load is a full [128, 4096] strip
            # (16KB contiguous per partition -> large DMA packets), with the 8
            # row-tiles accumulating in 8 parallel PSUM banks.
            with tc.tile_pool(name="pav", bufs=1) as pav:
                vT = pav.tile([128, ROWS], sb_dt)
                with tc.tile_pool(name="pa", bufs=3) as pa, \
                     tc.tile_pool(name="paw", bufs=1) as paw, \
                     tc.tile_pool(name="psa", bufs=1, space="PSUM") as psa:
                    w_sb = {}
                    for name, w in (("q", wq), ("k", wk), ("v", wv)):
                        t = paw.tile([128, KC, HD], io_dt, tag=f"w_{name}")
                        nc.sync.dma_start(t[:],
                                          w[:].rearrange("p (c l) -> p c l", c=KC))
                        w_sb[name] = t
                    dsts = {"q": qT, "k": kT, "v": vT}
                    srcs = {"q": xq, "k": xk, "v": xv}
                    MT = ROWS // 512
                    for ti, name in enumerate(("q", "k", "v")):
                        xr = srcs[name][:].rearrange("(c p) r -> p c r", p=128)
                        pss = [psa.tile([128, 512], F32, tag=f"proj{m}",
                                        name=f"ps_{name}_{m}")
                               for m in range(MT)]
                        for c in range(KC):
                            xs = pa.tile([128, ROWS], io_dt, tag="xstrip")
                            nc.sync.dma_start(xs[:], xr[:, c])
                            for m in range(MT):
                                nc.tensor.matmul(pss[m][:], mm(w_sb[name][:, c]),
                                                 mm(xs[:, m * 512:(m + 1) * 512]),
                                                 start=(c == 0),
                                                 stop=(c == KC - 1))
                        for m in range(MT):
                            dst = dsts[name][:, m * 512:(m + 1) * 512]
                            if with_bias:
                                nc.vector.tensor_scalar_add(dst, pss[m][:],
                                                            bias_sb[:, ti:ti + 1])
                            else:
                                nc.vector.tensor_copy(dst, pss[m][:])
                # ---- Phase B: transpose v into [kpos, d] layout.
                with tc.tile_pool(name="pstr", bufs=4, space="PSUM") as pstr:
                    for g in range(NG):
                        for j in range(JT):
                            pt = pstr.tile([128, 128], sb_dt, tag="tr")
                            base = g * SP + j * 128
                            nc.tensor.transpose(pt[:], vT[:, base:base + 128],
                                                ident[:])
                            nc.vector.tensor_copy(vN[:, g, j], pt[:])

            # ---- Phase C: attention per (batch, head) group.
            with tc.tile_pool(name="pc", bufs=2) as pc, \
                 tc.tile_pool(name="pss", bufs=4, space="PSUM") as pss, \
                 tc.tile_pool(name="pssum", bufs=2, space="PSUM") as pssum, \
                 tc.tile_pool(name="pso", bufs=2, space="PSUM") as pso:
                for g in range(NG):
                    b, hl = divmod(g, HPC)
                    Esb = pc.tile([128, JT, SP], sb_dt, tag="E")
                    for j in range(JT):
                        sp = pss.tile([128, SP], F32, tag="S")
                        base = g * SP + j * 128
                        nc.tensor.matmul(sp[:], mm(kT[:, base:base + 128]),
                                         mm(qT[:, g * SP:(g + 1) * SP]),
                                         start=True, stop=True)
                        nc.scalar.activation(Esb[:, j], sp[:],
                                             mybir.ActivationFunctionType.Exp)
                    sum_ps = pssum.tile([128, SP], F32, tag="sum")
                    for j in range(JT):
                        nc.tensor.matmul(sum_ps[:], mm(ones_t[:]), mm(Esb[:, j]),
                                         start=(j == 0), stop=(j == JT - 1))
                    o_ps = pso.tile([128, SP], F32, tag="O")
                    for j in range(JT):
                        nc.tensor.matmul(o_ps[:], mm(vN[:, g, j]), mm(Esb[:, j]),
                                         start=(j == 0), stop=(j == JT - 1))
                    # softmax denominator: reciprocal, then scale PV while
                    # draining its PSUM bank.
                    rec_b = pc.tile([128, SP], F32, tag="recb")
                    nc.vector.reciprocal(rec_b[:], sum_ps[:])
                    nc.vector.tensor_tensor(AT[:, hl, b * SP:(b + 1) * SP],
                                            o_ps[:], rec_b[:],
                                            op=mybir.AluOpType.mult)

            # ---- Phase D: partial out-projection (256 latent channels).
            with tc.tile_pool(name="pd", bufs=3) as pd, \
                 tc.tile_pool(name="pdw", bufs=1) as pdw, \
                 tc.tile_pool(name="psd", bufs=4, space="PSUM") as psd:
                wo_sb = pdw.tile([128, HPC, E], io_dt)
                nc.sync.dma_start(wo_sb[:],
                                  wo[:].rearrange("p (h e) -> p h e", h=HPC))
                for rt in range(OROWS // 128):
                    # stage the full 8KB output row per partition so the store
                    # is one large-packet DMA
                    ot = pd.tile([128, E], F32, tag="ot")
                    for n in range(E // 512):
                        ps = psd.tile([128, 512], F32, tag="od")
                        for hl in range(HPC):
                            nc.tensor.matmul(ps[:],
                                             mm(AT[:, hl, rt * 128:(rt + 1) * 128]),
                                             mm(wo_sb[:, hl, n * 512:(n + 1) * 512]),
                                             start=(hl == 0), stop=(hl == HPC - 1))
                        nc.vector.tensor_copy(ot[:, n * 512:(n + 1) * 512], ps[:])
                    nc.sync.dma_start(out[rt * 128:(rt + 1) * 128, :], ot[:])

    nc.compile()
    return nc


def _rope_matrix():
    h2 = HD // 2
    freqs = 1.0 / (10000.0 ** (np.arange(0, HD, 2, dtype=np.float64) / HD))
    sin, cos = np.sin(freqs), np.cos(freqs)
    R = np.zeros((HD, HD), np.float64)
    i = np.arange(h2)
    R[i, i] = cos
    R[i + h2, i] = -sin
    R[i + h2, i + h2] = cos
    R[i, i + h2] = sin
    return R


def kernel(query, key, value, attn_mask, Wq, bq, Wk, bk, Wv, bv, Wo, bo,
           _trace=False):
    global LAST_RESULTS
    mm_mode = MM_MODE
    io_np = np.dtype("float32")
    if mm_mode == "bf16":
        import ml_dtypes
        io_np = np.dtype(ml_dtypes.bfloat16)

    R = _rope_matrix()
    scale = 1.0 / np.sqrt(np.float64(HD))
    wq_eff = (Wq.astype(np.float64) @ R * scale).astype(io_np)
    wk_eff = (Wk.astype(np.float64) @ R).astype(io_np)
    wv_eff = Wv.astype(io_np)
    bq_eff = (bq.astype(np.float64) @ R * scale).astype(np.float32)
    bk_eff = (bk.astype(np.float64) @ R).astype(np.float32)
    bv_eff = bv.astype(np.float32)
    with_bias = bool(np.any(bq_eff) or np.any(bk_eff) or np.any(bv_eff))

    key_ = (mm_mode, with_bias)
    if key_ not in _CACHE:
        _CACHE[key_] = _build(mm_mode, with_bias)
    nc = _CACHE[key_]

    # [B,S,E] -> [E, B, H, SP]; s = s'*H + h so reshape(B, SP, H, E) puts the
    # folded position s' on axis 1 and the head on axis 2.
    def fold(x):
        return np.ascontiguousarray(
            x.reshape(B, SP, H, E).transpose(3, 0, 2, 1).astype(io_np))

    fq, fk, fv = fold(query), fold(key), fold(value)
    wo_r = Wo.reshape(H, HD, E)

    # pre-lay weights in SBUF order: [128 partitions, ...] with the partition
    # dim the *inner* 128 of the contraction index, so the on-device load is a
    # single contiguous-line DMA.
    def sb_layout_w(w_eff):  # [E, HD] -> [128, KC*HD]
        return np.ascontiguousarray(
            w_eff.reshape(KC, 128, HD).transpose(1, 0, 2).reshape(128, KC * HD))

    wq_sb, wk_sb, wv_sb = map(sb_layout_w, (wq_eff, wk_eff, wv_eff))

    in_maps = []
    for c in range(NCORES):
        h0 = HPC * c
        wo_c = wo_r[h0:h0 + HPC].astype(io_np)  # [HPC, HD, E]
        m = {
            "xq": fq[:, :, h0:h0 + HPC, :].reshape(E, ROWS),
            "xk": fk[:, :, h0:h0 + HPC, :].reshape(E, ROWS),
            "xv": fv[:, :, h0:h0 + HPC, :].reshape(E, ROWS),
            "wq": wq_sb, "wk": wk_sb, "wv": wv_sb,
            "wo": np.ascontiguousarray(
                wo_c.transpose(1, 0, 2).reshape(128, HPC * E)),
        }
        if with_bias:
            m["bqkv"] = np.stack([bq_eff, bk_eff, bv_eff])
        in_maps.append(m)

    kwargs = {}
    if _trace:
        kwargs = dict(trace=True, trace_cores=list(range(NCORES)))
    res = run_bass_kernel_spmd(nc, in_maps, core_ids=list(range(NCORES)),
                               **kwargs)
    LAST_RESULTS = res

    total = res.results[0]["out"].astype(np.float64)
    for c in range(1, NCORES):
        total += res.results[c]["out"]
    total += bo.astype(np.float64)
    return total.reshape(B, SP, E).astype(np.float32)
